# revision 39
# baseline (speedup 1.0000x reference)
"""Trainium2 Bass kernel for Chebyshev (L-inf) "convolution".

Math (see reference):
  out[b,co,h,w] = max_n |weights[co,n] - x_pad[b, c(co,n), h+di(co,n), w+dj(co,n)]| + bias[co]
  where conn_idx[co,n] = c*9 + di*3 + dj and x_pad is replicate-padded by 1.

Strategy (8 NeuronCores, batch-sharded: 4 images per core):
  1. Per image: load x contiguously into SBUF, build a replicate-padded
     bf16 plane set [C=64, 66*66] via an on-engine strided cast-copy, and
     store it contiguously to DRAM scratch xpad_b.
  2. Per (image, tap): one indirect DMA; output partition co reads a
     contiguous 4222-element span of xpad_b starting at element offset
     idx[co] = c*4356 + di*66 + dj (int32, one 8-int32 slot per partition,
     only col 0 used).  The shifted 64x64 window sits at row-stride 66
     inside the span.
  3. Per tap: T_n = |G_n - w_n| -> bf16 (ScalarE Abs-activation with
     per-partition bias=-w, or VectorE tensor_scalar (add -w, abs_max 0)).
  4. VectorE max tree over the 4 taps, + bias (fp32 out), DMA out.

Built with Bacc so multi-wait instructions are legalized into event
semaphores (TRN2 allows 1 sync-wait per instruction).
"""

import numpy as np

B, CIN, H, W = 32, 64, 64, 64
COUT, NCONN = 128, 4
KH, KW = 3, 3
NCORES = 8
BL = B // NCORES            # 4 images per core
PH, PW = H + 2, W + 2       # 66 x 66 replicate-padded planes
PLANE = PH * PW             # 4356
S = H * W                   # 4096
SPAN = (H - 1) * PW + W     # 4222: span holding one shifted 64x64 window
GPAD = SPAN + 2             # 4224 (even) SBUF tile width

_CACHE = {}


def _build_program():
    import concourse.bass as bass
    import concourse.bacc as bacc
    import concourse.mybir as mybir
    from concourse.tile import TileContext, add_dep_helper

    f32 = mybir.dt.float32
    bf16 = mybir.dt.bfloat16
    i32 = mybir.dt.int32
    Alu = mybir.AluOpType
    Act = mybir.ActivationFunctionType

    nc = bacc.Bacc("TRN2", target_bir_lowering=False, debug=False)

    x_ext = nc.dram_tensor("x", (BL, CIN, H, W), f32, kind="ExternalInput").ap()
    wneg_ext = nc.dram_tensor("wneg", (COUT, NCONN), f32, kind="ExternalInput").ap()
    bias_ext = nc.dram_tensor("bias", (COUT, 1), f32, kind="ExternalInput").ap()
    # per (b, n): one 8-int32 slot per partition at cols [(b*NCONN+n)*8, +8);
    # the indirect-DMA ucode reads col 0 (the rest pad the 32 B block).
    gidx_ext = nc.dram_tensor(
        "gidx", (COUT, BL * NCONN * 8), i32, kind="ExternalInput"
    ).ap()
    out_ext = [
        nc.dram_tensor(f"out{b}", (COUT, H, W), f32, kind="ExternalOutput").ap()
        for b in range(BL)
    ]
    xpads = [
        nc.dram_tensor(f"xpad{b}", (CIN * PLANE, 1), bf16) for b in range(BL)
    ]

    with TileContext(nc, pool_alloc_mode="queue") as tc:
        with (
            tc.tile_pool(name="const", bufs=1) as cpool,
            tc.tile_pool(name="xp", bufs=2) as xpool,
            tc.tile_pool(name="g", bufs=6) as gpool,
            tc.tile_pool(name="t", bufs=5) as tpool,
            tc.tile_pool(name="m", bufs=2) as mpool,
            tc.tile_pool(name="o", bufs=3) as opool,
        ):
            wneg_sb = cpool.tile([COUT, NCONN], f32)
            nc.sync.dma_start(out=wneg_sb[:], in_=wneg_ext)
            bias_sb = cpool.tile([COUT, 1], f32)
            nc.sync.dma_start(out=bias_sb[:], in_=bias_ext)
            gidx_sb = cpool.tile([COUT, BL * NCONN * 8], i32)
            nc.sync.dma_start(out=gidx_sb[:], in_=gidx_ext)

            for b in range(BL):
                # --- padded bf16 planes for image b ---
                XSB = xpool.tile([CIN, S], f32, tag="xsb")
                nc.sync.dma_start(
                    out=XSB[:], in_=x_ext[b].rearrange("c h w -> c (h w)")
                )
                XP = xpool.tile([CIN, PLANE], bf16, tag="xp")
                XPv = XP[:].rearrange("c (h w) -> c h w", h=PH, w=PW)
                nc.vector.tensor_copy(
                    out=XPv[:, 1 : H + 1, 1 : W + 1],
                    in_=XSB[:].rearrange("c (h w) -> c h w", h=H, w=W),
                )
                nc.vector.tensor_copy(
                    out=XPv[:, 1 : H + 1, 0:1], in_=XPv[:, 1 : H + 1, 1:2]
                )
                nc.vector.tensor_copy(
                    out=XPv[:, 1 : H + 1, PW - 1 : PW],
                    in_=XPv[:, 1 : H + 1, PW - 2 : PW - 1],
                )
                nc.vector.tensor_copy(out=XPv[:, 0:1, :], in_=XPv[:, 1:2, :])
                nc.vector.tensor_copy(
                    out=XPv[:, PH - 1 : PH, :], in_=XPv[:, PH - 2 : PH - 1, :]
                )
                # contiguous store of the padded planes (8.7 KiB/partition)
                store = nc.sync.dma_start(
                    out=xpads[b].ap().rearrange(
                        "(c p) one -> c (p one)", c=CIN, p=PLANE
                    ),
                    in_=XP[:],
                )

                # --- per tap: indirect span gather + |G - w| ---
                ts = []
                for n in range(NCONN):
                    k = b * NCONN + n
                    gt = gpool.tile([COUT, GPAD], bf16, tag="g")
                    gather = nc.gpsimd.indirect_dma_start(
                        out=gt[:, 0:SPAN],
                        out_offset=None,
                        in_=xpads[b].ap(),
                        in_offset=bass.IndirectOffsetOnAxis(
                            ap=gidx_sb[:, k * 8 : k * 8 + 1], axis=0
                        ),
                    )
                    add_dep_helper(
                        gather.ins, store.ins, reason="gather reads xpad[b]"
                    )
                    gv = gt[:].rearrange("p (h w) -> p h w", h=H, w=PW)[:, :, 0:W]
                    tt = tpool.tile([COUT, S], bf16, tag="t")
                    tv = tt[:].rearrange("p (h w) -> p h w", h=H, w=W)
                    nc.scalar.activation(
                        out=tv,
                        in_=gv,
                        func=Act.Abs,
                        bias=wneg_sb[:, n : n + 1],
                        scale=1.0,
                    )
                    ts.append(tt)

                # --- max tree + bias (VectorE), store out (SP HWDGE) ---
                m0 = mpool.tile([COUT, S], bf16, tag="m")
                nc.vector.tensor_tensor(
                    out=m0[:], in0=ts[0][:], in1=ts[1][:], op=Alu.max
                )
                m1 = mpool.tile([COUT, S], bf16, tag="m")
                nc.vector.tensor_tensor(
                    out=m1[:], in0=ts[2][:], in1=ts[3][:], op=Alu.max
                )
                # final max + bias + store pipelined at half-plane granularity
                outv = out_ext[b].rearrange("c h w -> c (h w)")
                for hh in range(2):
                    sl = slice(hh * (S // 2), (hh + 1) * (S // 2))
                    m2 = mpool.tile([COUT, S // 2], bf16, tag="m2")
                    nc.vector.tensor_tensor(
                        out=m2[:], in0=m0[:, sl], in1=m1[:, sl], op=Alu.max
                    )
                    ot = opool.tile([COUT, S // 2], f32, tag="o")
                    nc.vector.tensor_scalar(
                        out=ot[:],
                        in0=m2[:],
                        scalar1=bias_sb[:, 0:1],
                        scalar2=None,
                        op0=Alu.add,
                    )
                    nc.sync.dma_start(out=outv[:, sl], in_=ot[:])
    nc.compile()
    return nc


def _host_inputs(x, weights, bias, conn_idx):
    """Per-core input maps (host-side prep: shard x, derive -w / bias / gather
    row-indices from the tiny weight/index tensors)."""
    ci = np.asarray(conn_idx).astype(np.int64)          # [COUT, NCONN]
    c = ci // (KH * KW)
    rem = ci % (KH * KW)
    di = rem // KW
    dj = rem % KW
    # element offset into xpad_b [64, 66, 66]: c*4356 + di*66 + dj
    offs = (c * PLANE + di * PW + dj).astype(np.int32)          # [COUT, NCONN]
    gidx = np.zeros((COUT, BL * NCONN * 8), dtype=np.int32)
    for bb in range(BL):
        for n in range(NCONN):
            k = bb * NCONN + n
            gidx[:, k * 8] = offs[:, n]
    wneg = (-np.asarray(weights)).astype(np.float32)
    bias2 = np.asarray(bias).reshape(COUT, 1).astype(np.float32)
    x = np.ascontiguousarray(np.asarray(x), dtype=np.float32)
    in_maps = []
    for kcore in range(NCORES):
        in_maps.append(
            {
                "x": x[kcore * BL : (kcore + 1) * BL],
                "wneg": wneg,
                "bias": bias2,
                "gidx": gidx,
            }
        )
    return in_maps


def kernel(x, weights, bias, conn_idx):
    from concourse.bass_utils import run_bass_kernel_spmd

    if "nc" not in _CACHE:
        _CACHE["nc"] = _build_program()
    nc = _CACHE["nc"]
    in_maps = _host_inputs(x, weights, bias, conn_idx)
    res = run_bass_kernel_spmd(nc, in_maps, list(range(NCORES)))
    outs = [
        np.stack([np.asarray(res.results[k][f"out{b}"]) for b in range(BL)])
        for k in range(NCORES)
    ]
    return np.concatenate(outs, axis=0).astype(np.float32)


if __name__ == "__main__":
    nc = _build_program()
    print("program built OK")
